# revision 30
# baseline (speedup 1.0000x reference)
"""DecoupledIKLoss Trainium2 kernel (8-core data-parallel), v4.

Math: the reference computes ang = atan2(s_raw, c_raw) (degrees) then sin/cos
of those angles inside DH matrices.  Since sin(atan2(s,c)) = s/sqrt(s^2+c^2)
the FK chain collapses to a closed form in t = tanh(pred_raw):

    q_j  = a_j^2 + b_j^2            (a = t[:, 2j], b = t[:, 2j+1])
    inv_j = rsqrt(q_j) = exp(-0.5 ln q_j)
    s_j, c_j = a_j inv_j, b_j inv_j
    s23 = s2 c3 + c2 s3 ; c23 = c2 c3 - s2 s3
    u   = L (c2 + s23) - A3 c23
    P5x = c1 u - d2 s1 ;  P5y = s1 u + d2 c1
    P5z = L (c23 - s2) + A3 s23 + D1

    loss = mean((t - g)^2) + 2 mean(((P5 - X)/R)^2) + 0.05 mean((q - 1)^2)

All three loss terms are reduced via quadratic expansion so the raw target
tensors only ever feed the PE array:

    sc   = [Sum t^2] + [Sum g^2] - 2 [Sum t.g]
    circ = [Sum q^2] - 2 [Sum q] + 3N      with  Sum q == Sum t^2  (free!)
    wc   = [Sum u^2 + Sum P5z^2 + N d2^2]  - 2 [Sum P5.X] + [Sum X^2]
           (P5x^2 + P5y^2 = u^2 + d2^2 exactly, since s1^2+c1^2 = 1 --
            P5x, P5y are never materialized)
    Sum P5.X = Sum(u c1) Xx + Sum(u s1) Xy + d2 (Sum c1 Xy - Sum s1 Xx)
               + Sum P5z' Xz + D1 Sum Xz          (P5z' = P5z - D1)

Engine allocation per tile (8 tiles of [128 part x 512 rows] per core):
  - DMA  (SWDGE cast): pred f32->bf16, p5 f32->bf16, targ f32->fp8e4
    (targets only feed matmuls; fp8 quantization of uniform[-1,1] data
    biases Sum g^2 by ~5e-4 relative, ~1e-4 on the loss)
  - PE:  Sum g^2 as fp8 DoubleRow "diagonal" matmuls (2 rows/cycle);
         Sum t.g as mixed bf16 x fp8 cross matmuls; Sum q^2 / u^2 / P5z'^2
         / X^2 as bf16 diagonal matmuls; the 5 wc cross terms vs strided
         interleaved X; Sum t^2 / Sum P5z' / Sum Xz as ones-vector matmuls
  - ACT: tanh (strided -> per-joint block layout), ln, exp.  The exp is
         split per channel so joint 2's rsqrt absorbs the scale L via the
         activation bias: exp(-0.5 ln q + ln L) = L/sqrt(q).  tanh and
         ln/exp live in different ACT table sets, so tiles are processed
         in pairs ([tanh tanh][ln exp ln exp]) to halve table reloads.
  - DVE: the FK elementwise chain in bf16 tensor_tensor (2x mode) and
         tensor_scalar (4x mode) ops only -- scalar_tensor_tensor turned
         out to run at 1x and is avoided entirely.
"""

import numpy as np

N_CORES = 8
B = 4194304
BS = B // N_CORES            # rows per core
P = 128                      # SBUF partitions
W = 512                      # rows per partition per tile
NT = BS // (P * W)           # 8 tiles
assert NT * P * W == BS and NT % 2 == 0

_L = 431.8                   # DH a2 (== D4 wrist offset)
_A3 = 20.32                  # |DH a3|
_D2 = 139.7                  # DH d2
_D1 = 671.83                 # DH d1
_R = 900.0                   # workspace radius
_K = _A3 / _L                # A3/L
# fold the circ Sum q^2 into the wc diagonal accumulator: scale Q by
# lam = sqrt(w_circ / w_wc) so  w_wc * lam^2 * Sum q^2 == w_circ * Sum q^2
_LAM = float(np.sqrt(0.05 * _R * _R / 2.0))

_BUILT = None
TRACE = False
LAST_EXEC_NS = None
LAST_TRACE_PATH = None


def _build():
    import math
    import concourse.tile as tile
    from concourse import bacc, mybir

    f32 = mybir.dt.float32
    bf16 = mybir.dt.bfloat16
    f8e4 = mybir.dt.float8e4
    Act = mybir.ActivationFunctionType
    Alu = mybir.AluOpType
    DR = mybir.MatmulPerfMode.DoubleRow

    nc = bacc.Bacc("TRN2", target_bir_lowering=False, debug=False,
                   num_devices=N_CORES)

    pred = nc.dram_tensor("pred_raw", [BS, 6], f32, kind="ExternalInput")
    targ = nc.dram_tensor("target_sc", [BS, 6], f32, kind="ExternalInput")
    p5t = nc.dram_tensor("p5_target", [BS, 3], f32, kind="ExternalInput")
    out = nc.dram_tensor("out", [P, 6, 128], f32, kind="ExternalOutput")
    out3 = nc.dram_tensor("out3", [1, 1024], f32, kind="ExternalOutput")

    ones_bf = nc.alloc_sbuf_tensor("ones_bf", [P, 1], bf16)
    nc.gpsimd.memset(ones_bf.ap(), 1.0)
    lnl_sb = nc.alloc_sbuf_tensor("lnl_sb", [P, 1], f32)
    nc.gpsimd.memset(lnl_sb.ap(), math.log(_L))
    nc.all_engine_barrier()

    with tile.TileContext(nc) as tc:
        with (
            tc.tile_pool(name="inp", bufs=4) as inp,
            tc.tile_pool(name="jp", bufs=3) as jp,
            tc.tile_pool(name="wk", bufs=2) as wk,
            tc.tile_pool(name="psum", bufs=1, space="PSUM") as psum,
            tc.tile_pool(name="fin", bufs=1) as fin,
        ):
            ps_g = psum.tile([P, 128], f32)     # Sum g^2 (fp8 DR diag)
            ps_jg = psum.tile([P, 128], f32)    # Sum t*g (mixed cross)
            ps_wc = psum.tile([P, 128], f32)    # u^2+P5z'^2+X^2+lam^2 q^2
            ps_w1 = psum.tile([P, 128], f32)    # Sum UC1.Xx + US1.Xy + P5z'.Xz
            ps_w2 = psum.tile([P, 128], f32)    # Sum c1.Xy
            ps_w3 = psum.tile([P, 128], f32)    # Sum s1.Xx
            ps_t2 = psum.tile([1, 512], f32)    # ones @ SQ   (= Sum t^2 = Sum q)
            ps_row = psum.tile([1, 512], f32)   # [0:256] ones@P5z' | [256:512] ones@Xz

            cnt = {"g": 0, "jg": 0, "wc": 0, "w1": 0, "w2": 0,
                   "w3": 0, "t2": 0, "z": 0, "xz": 0}
            tot = {"g": NT * 12, "jg": NT * 24,
                   "wc": NT * 32, "w1": NT * 12, "w2": NT * 4,
                   "w3": NT * 4, "t2": NT * 6, "z": NT * 2, "xz": NT * 2}
            accs = {"g": ps_g[:], "jg": ps_jg[:], "wc": ps_wc[:],
                    "w1": ps_w1[:], "w2": ps_w2[:], "w3": ps_w3[:],
                    "t2": ps_t2[:], "z": ps_row[0:1, 0:256],
                    "xz": ps_row[0:1, 256:512]}

            def mm(which, lhsT, rhs, perf_mode=None):
                i = cnt[which]
                cnt[which] += 1
                nc.tensor.matmul(
                    accs[which], lhsT, rhs,
                    start=(i == 0), stop=(i == tot[which] - 1),
                    perf_mode=perf_mode, skip_group_check=True,
                )

            KC = W // 128
            LNL = lnl_sb.ap()

            def load_tile(t):
                rows = P * W
                row0 = t * rows
                pred_v = pred.ap()[row0:row0 + rows, :].rearrange(
                    "(p w) c -> p (w c)", p=P)
                targ_v = targ.ap()[row0:row0 + rows, :].rearrange(
                    "(p w) c -> p (w c)", p=P)
                p5t_v = p5t.ap()[row0:row0 + rows, :].rearrange(
                    "(p w) c -> p (w c)", p=P)
                pred_sb = inp.tile([P, W, 6], bf16, tag="pred")
                nc.gpsimd.dma_start(out=pred_sb, in_=pred_v)
                tg8 = inp.tile([P, W, 6], f8e4, tag="tg")
                nc.gpsimd.dma_start(out=tg8, in_=targ_v)
                x_sb = inp.tile([P, W, 3], bf16, tag="x")
                nc.gpsimd.dma_start(out=x_sb, in_=p5t_v)
                return pred_sb, tg8, x_sb

            def input_mms(tg8, x_sb):
                # PE self terms on raw inputs -- fire as soon as loads land
                g8v = tg8.rearrange("p a b -> p (a b)").rearrange(
                    "p (m two n) -> p m two n", two=2, n=128)
                for i in range(6 * KC // 2):
                    mm("g", g8v[:, i], g8v[:, i], perf_mode=DR)
                xf = x_sb.rearrange("p a b -> p (a b)")
                for k in range(3 * KC):
                    sl = slice(k * 128, (k + 1) * 128)
                    mm("wc", xf[:, sl], xf[:, sl])
                for k in range(2):
                    sl = slice(k * 256, (k + 1) * 256)
                    mm("xz", ones_bf.ap(), x_sb[:, sl, 2])

            def tanh_tile(t, pred_sb):
                J = jp.tile([P, 6, W], bf16, tag="J")
                nc.scalar.activation(out=J, in_=pred_sb.transpose([0, 2, 1]),
                                     func=Act.Tanh)
                return J

            def body(t, J, tg8, x_sb):
                # PE: sc cross term, bf16 lhsT x fp8 strided rhs
                for j in range(6):
                    for k in range(KC):
                        sl = slice(k * 128, (k + 1) * 128)
                        mm("jg", J[:, j, sl], tg8[:, sl, j])

                # DVE: SQ = t^2 ; q = a^2 + b^2
                SQ = wk.tile([P, 6, W], bf16, tag="SQ")
                nc.vector.tensor_mul(SQ, J, J)
                sqf = SQ.rearrange("p a b -> p (a b)")
                for k in range((6 * W) // 512):
                    mm("t2", ones_bf.ap(), sqf[:, k * 512:(k + 1) * 512])
                Q = wk.tile([P, 3, W], bf16, tag="Q")
                nc.vector.tensor_add(Q, SQ[:, 0::2, :], SQ[:, 1::2, :])
                QS = wk.tile([P, 3, W], bf16, tag="QS")
                nc.vector.tensor_scalar_mul(QS, Q, _LAM)
                qf = QS.rearrange("p a b -> p (a b)")
                for k in range(3 * KC):
                    sl = slice(k * 128, (k + 1) * 128)
                    mm("wc", qf[:, sl], qf[:, sl])

                # ACT: inv = exp(-0.5 ln q); joint 2 gets bias ln(L)
                LQ = wk.tile([P, 3, W], bf16, tag="LQ")
                nc.scalar.activation(out=LQ, in_=Q, func=Act.Ln)
                INV = wk.tile([P, 3, W], bf16, tag="INV")
                nc.scalar.activation(out=INV[:, 0::2, :], in_=LQ[:, 0::2, :],
                                     func=Act.Exp, scale=-0.5)
                nc.scalar.activation(out=INV[:, 1, :], in_=LQ[:, 1, :],
                                     func=Act.Exp, scale=-0.5, bias=LNL)

                # DVE: SC = [s1 | c1 | L s2 | L c2 | s3 | c3]
                SC = wk.tile([P, 6, W], bf16, tag="SC")
                j4 = J.rearrange("p (j k) w -> p j k w", k=2)
                sc4 = SC.rearrange("p (j k) w -> p j k w", k=2)
                invb = INV.unsqueeze(2).broadcast_to([P, 3, 2, W])
                nc.vector.tensor_mul(sc4, j4, invb)

                s2c2 = SC[:, 2:4, :]               # [L s2 | L c2]
                s3b = SC[:, 4:5, :].broadcast_to([P, 2, W])
                c3b = SC[:, 5:6, :].broadcast_to([P, 2, W])

                MP1 = wk.tile([P, 2, W], bf16, tag="MP1")  # [L s2c3 | L c2c3]
                nc.vector.tensor_mul(MP1, s2c2, c3b)
                MP2 = wk.tile([P, 2, W], bf16, tag="MP2")  # [L s2s3 | L c2s3]
                nc.vector.tensor_mul(MP2, s2c2, s3b)

                SCC = wk.tile([P, 2, W], bf16, tag="SCC")  # [L s23 | L c23]
                nc.vector.tensor_add(SCC[:, 0, :], MP1[:, 0, :], MP2[:, 1, :])
                nc.vector.tensor_sub(SCC[:, 1, :], MP1[:, 1, :], MP2[:, 0, :])
                s23L = SCC[:, 0, :]
                c23L = SCC[:, 1, :]

                # KSC = k * [L s23 | L c23] = [A3 s23 | A3 c23]
                KSC = wk.tile([P, 2, W], bf16, tag="KSC")
                nc.vector.tensor_scalar_mul(KSC, SCC, _K)

                T1L = wk.tile([P, W], bf16, tag="T1L")   # L (c2 + s23)
                nc.vector.tensor_add(T1L, SC[:, 3, :], s23L)
                ZZ = wk.tile([P, W], bf16, tag="ZZ")     # L (c23 - s2)
                nc.vector.tensor_sub(ZZ, c23L, SC[:, 2, :])

                U = wk.tile([P, W], bf16, tag="U")       # u
                nc.vector.tensor_sub(U, T1L, KSC[:, 1, :])
                P5Z = wk.tile([P, W], bf16, tag="P5Z")   # P5z - D1
                nc.vector.tensor_add(P5Z, ZZ, KSC[:, 0, :])
                for k in range(2):
                    sl = slice(k * 256, (k + 1) * 256)
                    mm("z", ones_bf.ap(), P5Z[:, sl])

                UCUS = wk.tile([P, 2, W], bf16, tag="UCUS")  # [u s1 | u c1]
                ub = U.unsqueeze(1).broadcast_to([P, 2, W])
                nc.vector.tensor_mul(UCUS, SC[:, 0:2, :], ub)

                # PE: wc terms
                for k in range(KC):
                    sl = slice(k * 128, (k + 1) * 128)
                    mm("wc", U[:, sl], U[:, sl])
                    mm("wc", P5Z[:, sl], P5Z[:, sl])
                    mm("w1", UCUS[:, 1, sl], x_sb[:, sl, 0])
                    mm("w1", UCUS[:, 0, sl], x_sb[:, sl, 1])
                    mm("w1", P5Z[:, sl], x_sb[:, sl, 2])
                    mm("w2", SC[:, 1, sl], x_sb[:, sl, 1])
                    mm("w3", SC[:, 0, sl], x_sb[:, sl, 0])

            # software-pipelined pair loop: ACT order is
            # [tanh 2p, tanh 2p+1], [ln/exp 2p, ln/exp 2p+1], ...
            staged = {}
            for t in range(2):
                staged[t] = load_tile(t)
            for p in range(NT // 2):
                t0, t1 = 2 * p, 2 * p + 1
                pred0, tg0, x0 = staged.pop(t0)
                pred1, tg1, x1 = staged.pop(t1)
                input_mms(tg0, x0)
                J0 = tanh_tile(t0, pred0)
                input_mms(tg1, x1)
                J1 = tanh_tile(t1, pred1)
                for tn in (2 * p + 2, 2 * p + 3):
                    if tn < NT:
                        staged[tn] = load_tile(tn)
                body(t0, J0, tg0, x0)
                body(t1, J1, tg1, x1)

            for k, v in cnt.items():
                assert v == tot[k], (k, v, tot[k])

            # ---- epilogue: PSUM -> SBUF -> DRAM ----
            ob = fin.tile([P, 6, 128], f32)
            nc.vector.tensor_copy(ob[:, 0, :], ps_g[:])
            nc.vector.tensor_copy(ob[:, 1, :], ps_jg[:])
            nc.vector.tensor_copy(ob[:, 2, :], ps_wc[:])
            nc.vector.tensor_copy(ob[:, 3, :], ps_w1[:])
            nc.vector.tensor_copy(ob[:, 4, :], ps_w2[:])
            nc.vector.tensor_copy(ob[:, 5, :], ps_w3[:])
            nc.sync.dma_start(out.ap(), ob[:])
            ob3 = fin.tile([1, 1024], f32)
            nc.vector.tensor_copy(ob3[0:1, 0:512], ps_t2[:])
            nc.vector.tensor_copy(ob3[0:1, 512:1024], ps_row[:])
            nc.sync.dma_start(out3.ap(), ob3[:])

    nc.compile()
    return nc


def _get_built():
    global _BUILT
    if _BUILT is None:
        _BUILT = _build()
    return _BUILT


def kernel(pred_raw, target_sc, P5_target):
    global LAST_EXEC_NS, LAST_TRACE_PATH
    import jax
    if jax.config.jax_platforms != "axon":
        jax.config.update("jax_platforms", "axon")
    from concourse.bass_utils import run_bass_kernel_spmd

    pred_raw = np.ascontiguousarray(pred_raw, dtype=np.float32)
    target_sc = np.ascontiguousarray(target_sc, dtype=np.float32)
    P5_target = np.ascontiguousarray(P5_target, dtype=np.float32)

    nc = _get_built()
    in_maps = []
    for c in range(N_CORES):
        sl = slice(c * BS, (c + 1) * BS)
        in_maps.append({
            "pred_raw": pred_raw[sl],
            "target_sc": target_sc[sl],
            "p5_target": P5_target[sl],
        })
    res = run_bass_kernel_spmd(nc, in_maps, core_ids=list(range(N_CORES)),
                               trace=TRACE)
    LAST_EXEC_NS = res.exec_time_ns
    LAST_TRACE_PATH = (None if res.instructions_and_trace is None
                       else res.instructions_and_trace[1])

    sc = np.float64(0.0)
    wcq = np.float64(0.0)   # wc numerator, with lam^2 Sum q^2 folded in
    circ_lin = np.float64(0.0)
    n = np.float64(BS)
    for c in range(N_CORES):
        o = res.results[c]["out"].astype(np.float64)     # [P, 6, 128]
        r3 = res.results[c]["out3"].astype(np.float64).ravel()  # [1024]
        tr = np.einsum('pip->i', o)                      # traces
        s_t2q = r3[0:512].sum()
        s_z = r3[512:768].sum()
        s_xz = r3[768:1024].sum()
        sc += s_t2q + tr[0] - 2.0 * tr[1]
        circ_lin += -2.0 * s_t2q + 3.0 * n
        # tr[2] = Sum u^2 + Sum P5z'^2 + Sum X^2 + lam^2 Sum q^2
        p5p5x2 = tr[2] + 2.0 * _D1 * (s_z - s_xz) + n * (_D1 * _D1 + _D2 * _D2)
        p5x_rest = tr[3] + _D2 * (tr[4] - tr[5])
        wcq += p5p5x2 - 2.0 * p5x_rest

    loss = (sc / (6 * B)
            + 2.0 * wcq / (_R * _R * 3 * B)
            + 0.05 * circ_lin / (3 * B))
    return np.asarray(np.float32(loss))
